# revision 11
# baseline (speedup 1.0000x reference)
"""nn_MultiHeadAttention: fused MHA + residual + LayerNorm on 8 TRN2 NeuronCores.

Sharding: core = (batch b, query-half). Each core computes, for its batch:
  - Q projection for its 512 query rows, K/V projections for all 1024 keys
    (K/V work duplicated within a batch pair -> zero cross-core communication),
  - all 16 heads' attention for its query rows,
  - output projection + residual + LayerNorm for its rows.
Host concatenates the 8 [512, 1024] results into [4, 1024, 1024].

Schedule: K/Q projections are split into jg0 (head pairs 0-3) and jg1
(pairs 4-7) column-group phases.  Attention for pairs 0-3 (scores row-packed
2 heads across the PE's 64-row groups -> concurrent; exp on ACT; PV) starts
right after jg0 and overlaps the jg1 projections, the four V passes and the
jg1 attention, so the ~70us of ACT exp work hides behind PE matmuls.

Engine/queue split: SP-HWDGE queue carries weights (as [128, 512] col-half
chunks) + output; ACT-HWDGE carries the pre-attention xk/xq loads; the
gpsimd SWDGE ring carries every load issued while ACT is busy with exp
(xv, the jg1 xk/xq re-loads, resid).  ACT does exp + sqrt only; all
PSUM->SBUF copies run on DVE.  All matmuls f32r (1 cycle/row at N=512).
"""
import numpy as np

import concourse.bass as bass
import concourse.mybir as mybir
import concourse.tile as tile
from concourse import bacc, bass_utils

B, S, D, H, DK = 4, 1024, 1024, 16, 64
P = 128
SH = S // 2           # query rows per core
NC = D // P           # 8 chunks of 128 along any d-dimension
NP = H // 2           # 8 head pairs (one 128-dim chunk each)
NCORES = 8
EPS = 1e-6
f32 = mybir.dt.float32
f32r = mybir.dt.float32r

TRACE = False          # set by test.py to profile
LAST_EXEC_NS = None

_CACHE = {}


def _build():
    nc = bacc.Bacc("TRN2")
    xqT = nc.dram_tensor("xqT", [D, SH], f32, kind="ExternalInput")
    xkT = nc.dram_tensor("xkT", [D, S], f32, kind="ExternalInput")
    xvT = nc.dram_tensor("xvT", [D, S], f32, kind="ExternalInput")
    wq = nc.dram_tensor("wq", [D, D], f32, kind="ExternalInput")   # Wq.T  [in, out]
    wk = nc.dram_tensor("wk", [D, D], f32, kind="ExternalInput")
    wv = nc.dram_tensor("wv", [D, D], f32, kind="ExternalInput")
    wo = nc.dram_tensor("wo", [D, D], f32, kind="ExternalInput")   # Wo.T  [d, e]
    resid = nc.dram_tensor("resid", [SH, D], f32, kind="ExternalInput")
    gamma = nc.dram_tensor("gamma", [D], f32, kind="ExternalInput")
    beta = nc.dram_tensor("beta", [D], f32, kind="ExternalInput")
    out = nc.dram_tensor("out", [SH, D], f32, kind="ExternalOutput")

    with tile.TileContext(nc) as tc:
        with (
            tc.tile_pool(name="wpool", bufs=20) as wpool,
            tc.tile_pool(name="xs", bufs=4) as xsp,
            tc.tile_pool(name="xvp", bufs=7) as xvp,
            tc.tile_pool(name="persist", bufs=1) as persist,
            tc.tile_pool(name="expp", bufs=5) as expp,
            tc.tile_pool(name="rp", bufs=2) as rp,
            tc.tile_pool(name="small", bufs=2) as small,
            tc.tile_pool(name="psum", bufs=3, space="PSUM") as psum,
            tc.tile_pool(name="pvps", bufs=2, space="PSUM") as pvps,
        ):
            # ---------------- persistent tiles ----------------
            kT = persist.tile([P, NC, S], f32r)       # [dim-in-pair, pair, sk]
            qT = persist.tile([P, NC, SH], f32r)      # [dim-in-pair, pair, sq]
            vt = persist.tile([P, NC, H, DK + 1], f32r)  # [sk-in-chunk, sk-chunk, (h, d|1)]
            xT = persist.tile([P, NC, SH], f32r)      # normalized attn out
            gb = persist.tile([P, 2, D], f32)         # gamma/beta broadcast
            eps_t = persist.tile([P, 1], f32)

            nc.vector.memset(eps_t, EPS)
            nc.vector.memset(vt[:, :, :, DK:DK + 1].bitcast(f32), 1.0)  # ones col

            def load_wh(w, i, ch, nm):
                """One [128, 512] weight col-half chunk (SP queue)."""
                wt = wpool.tile([P, SH], f32r, tag="w", name=f"{nm}{i}")
                nc.sync.dma_start(
                    wt, w[i * P:(i + 1) * P, ch * SH:(ch + 1) * SH].bitcast(f32r)
                )
                return wt

            def load_x(eng, pool, x, i, col0, nm):
                """One [128, 512] activation half-chunk (ACT or SWDGE queue)."""
                xc = pool.tile([P, SH], f32r, tag="x", name=nm)
                eng.dma_start(xc, x[i * P:(i + 1) * P, col0:col0 + SH].bitcast(f32r))
                return xc

            def big():
                return psum.tile([P, 2, SH], f32, tag="mm", name="big")

            # ---------------- K / Q projection passes ----------------
            # kq_pass computes out_cols jg*512..jg*512+511 of the projection
            # (head pairs 4jg..4jg+3) for source columns [half].
            def kq_pass(w8, xget, jg, dst, dsl):
                ps2 = [big() for _ in range(2)]
                for i in range(NC):
                    xc = xget(i)
                    for jj in range(4):
                        nc.tensor.matmul(
                            ps2[jj // 2][:, jj % 2, :],
                            w8[i][:, jj * P:(jj + 1) * P], xc,
                            start=(i == 0), stop=(i == NC - 1),
                        )
                for jj in range(4):
                    nc.vector.tensor_copy(dst[:, jg * 4 + jj, dsl], ps2[jj // 2][:, jj % 2, :])

            def k_passes(jg, reload):
                w8 = [load_wh(wk, i, jg, "wk") for i in range(NC)]
                for half in range(2):
                    xk8 = {}

                    def xget(i, half=half, xk8=xk8):
                        if i not in xk8:
                            eng, pool = (nc.gpsimd, xvp) if reload else (nc.scalar, xsp)
                            xk8[i] = load_x(eng, pool, xkT, i, half * SH, "xk")
                        return xk8[i]

                    kq_pass(w8, xget, jg, kT, slice(half * SH, (half + 1) * SH))

            def q_pass(jg, reload):
                w8 = [load_wh(wq, i, jg, "wq") for i in range(NC)]
                xq8 = {}

                def xget(i):
                    if i not in xq8:
                        eng, pool = (nc.gpsimd, xvp) if reload else (nc.scalar, xsp)
                        xq8[i] = load_x(eng, pool, xqT, i, 0, "xq")
                    return xq8[i]

                kq_pass(w8, xget, jg, qT, slice(None))

            # ---------------- V projection passes ----------------
            wv_half = {}

            def v_pass(scg, dh):
                if dh not in wv_half:
                    wv_half[dh] = [load_wh(wv, i, dh, "wv") for i in range(NC)]
                ps_v = [big() for _ in range(2)]
                for i in range(NC):
                    xc = load_x(nc.gpsimd, xvp, xvT, i, scg * SH, "xv")
                    for sl in range(4):
                        nc.tensor.matmul(
                            ps_v[sl // 2][:, sl % 2, :],
                            xc[:, sl * P:(sl + 1) * P], wv_half[dh][i],
                            start=(i == 0), stop=(i == NC - 1),
                        )
                for sl in range(4):
                    sc = scg * 4 + sl
                    nc.vector.tensor_copy(
                        vt[:, sc, dh * 8:(dh + 1) * 8, :DK],
                        ps_v[sl // 2][:, sl % 2, :].rearrange("p (h d) -> p h d", d=DK),
                    )

            # ---------------- attention ----------------
            pv_state = {}

            def sc_exp_cp(p, cp):
                """Row-packed scores for heads (2p, 2p+1), sk chunks
                (2cp, 2cp+1), then exp. Returns (etA, etB)."""
                ps2 = [big() for _ in range(2)]
                for k in range(2):
                    c = 2 * cp + k
                    for a in range(2):
                        nc.tensor.matmul(
                            ps2[a][:, k, :],
                            kT[a * DK:(a + 1) * DK, p, c * P:(c + 1) * P],
                            qT[a * DK:(a + 1) * DK, p, :],
                            start=True, stop=True,
                        )
                ets = []
                for a in range(2):
                    et = expp.tile([P, 2, SH], f32r, tag="e", name="et")
                    nc.scalar.activation(
                        out=et, in_=ps2[a],
                        func=mybir.ActivationFunctionType.Exp,
                        scale=1.0 / np.sqrt(np.float32(DK)),
                    )
                    ets.append(et)
                return ets

            def pv_cp(h, cp, et):
                """Accumulate PV for head h over sk chunks (2cp, 2cp+1);
                normalize after the last chunk."""
                if h not in pv_state:
                    pv_state[h] = pvps.tile([P, SH], f32, tag="pv", name="pvt")
                pvt = pv_state[h]
                for k in range(2):
                    c = 2 * cp + k
                    nc.tensor.matmul(
                        pvt[:DK + 1, :], vt[:, c, h, :], et[:, k, :],
                        start=(c == 0), stop=(c == NC - 1),
                    )
                if cp != 3:
                    return
                del pv_state[h]
                sums_raw = small.tile([1, SH], f32, tag="sums_raw", name="sums_raw",
                                      bufs=1)
                nc.vector.tensor_copy(sums_raw, pvt[DK:DK + 1, :])
                sums = small.tile([1, SH], f32, tag="sums", name="sums")
                # approx reciprocal needs an SBUF input (bit-trick path)
                nc.vector.reciprocal_approx_fast(sums, sums_raw)
                rbc = small.tile([DK, SH], f32, tag="rbc", name="rbc")
                nc.gpsimd.partition_broadcast(rbc, sums)
                a, p = h % 2, h // 2
                nc.vector.tensor_mul(
                    out=xT[a * DK:(a + 1) * DK, p, :], in0=pvt[:DK, :], in1=rbc
                )

            def pair(p, cps):
                for cp in cps:
                    etA, etB = sc_exp_cp(p, cp)
                    pv_cp(2 * p, cp, etA)
                    pv_cp(2 * p + 1, cp, etB)

            # jg0 projections, then attention on pairs 0-3 with the V passes
            # and jg1 projections interleaved to keep the PE dense under exp.
            k_passes(0, reload=False)
            q_pass(0, reload=False)
            e00 = sc_exp_cp(0, 0)
            e01 = sc_exp_cp(0, 1)
            v_pass(0, 0)
            for cp, (eA, eB) in ((0, e00), (1, e01)):
                pv_cp(0, cp, eA)
                pv_cp(1, cp, eB)
            e02 = sc_exp_cp(0, 2)
            e03 = sc_exp_cp(0, 3)
            v_pass(1, 0)
            for cp, (eA, eB) in ((2, e02), (3, e03)):
                pv_cp(0, cp, eA)
                pv_cp(1, cp, eB)
            pair(1, range(4))
            pair(2, range(4))
            pair(3, range(4))
            k_passes(1, reload=True)
            q_pass(1, reload=True)
            v_pass(0, 1)
            pair(4, range(4))
            pair(5, range(4))
            v_pass(1, 1)
            pair(6, range(4))
            pair(7, range(4))

            # ---------------- output projection + residual + LN ----------
            wo16 = [[load_wh(wo, i, eh, "wo") for eh in range(2)] for i in range(NC)]
            for i, t in enumerate((gamma, beta)):
                nc.gpsimd.dma_start(
                    gb[:, i, :], bass.AP(tensor=t, offset=0, ap=[[0, P], [1, D]])
                )
            for scc in range(4):
                rc = rp.tile([P, D], f32, tag="r", name="rc")
                nc.gpsimd.dma_start(rc, resid[scc * P:(scc + 1) * P, :])
                ps_o = big()
                for dc in range(NC):
                    for eh in range(2):
                        nc.tensor.matmul(
                            ps_o[:, eh, :],
                            xT[:, dc, scc * P:(scc + 1) * P],
                            wo16[dc][eh],
                            start=(dc == 0), stop=(dc == NC - 1),
                        )
                xl = rc  # LN runs in-place on the residual tile
                nc.vector.tensor_add(
                    out=xl, in0=ps_o.rearrange("p a b -> p (a b)"), in1=rc
                )
                stats = small.tile([P, 2, nc.vector.BN_STATS_DIM], f32, tag="stats",
                                   name="stats")
                for i in range(2):
                    nc.vector.bn_stats(stats[:, i, :], xl[:, i * SH:(i + 1) * SH])
                mv = small.tile([P, nc.vector.BN_AGGR_DIM], f32, tag="mv", name="mv")
                nc.vector.bn_aggr(mv, stats)
                std = small.tile([P, 1], f32, tag="std", name="std")
                nc.scalar.activation(
                    out=std, in_=mv[:, 1:2],
                    func=mybir.ActivationFunctionType.Sqrt,
                    bias=eps_t, scale=1.0,
                )
                rstd = small.tile([P, 1], f32, tag="rstd", name="rstd")
                nc.vector.reciprocal_approx_fast(rstd, std)
                nc.vector.tensor_scalar(
                    out=xl, in0=xl, scalar1=mv[:, 0:1], scalar2=rstd,
                    op0=mybir.AluOpType.subtract, op1=mybir.AluOpType.mult,
                )
                nc.vector.tensor_mul(out=xl, in0=xl, in1=gb[:, 0, :])
                nc.vector.tensor_add(out=xl, in0=xl, in1=gb[:, 1, :])
                nc.sync.dma_start(out[scc * P:(scc + 1) * P, :], xl)

    nc.compile()
    return nc


def kernel(query, key, value, Wq, Wk, Wv, Wo, ln_gamma, ln_beta):
    global LAST_EXEC_NS
    if "nc" not in _CACHE:
        _CACHE["nc"] = _build()
    nc = _CACHE["nc"]

    query = np.asarray(query, np.float32)
    key = np.asarray(key, np.float32)
    value = np.asarray(value, np.float32)
    wqT = np.ascontiguousarray(np.asarray(Wq, np.float32).T)
    wkT = np.ascontiguousarray(np.asarray(Wk, np.float32).T)
    wvT = np.ascontiguousarray(np.asarray(Wv, np.float32).T)
    woT = np.ascontiguousarray(np.asarray(Wo, np.float32).T)
    gamma = np.ascontiguousarray(np.asarray(ln_gamma, np.float32))
    beta = np.ascontiguousarray(np.asarray(ln_beta, np.float32))

    in_maps = []
    for core in range(NCORES):
        b, half = core // 2, core % 2
        sl = slice(half * SH, (half + 1) * SH)
        in_maps.append({
            "xqT": np.ascontiguousarray(query[b].T[:, sl]),
            "xkT": np.ascontiguousarray(key[b].T),
            "xvT": np.ascontiguousarray(value[b].T),
            "wq": wqT, "wk": wkT, "wv": wvT, "wo": woT,
            "resid": np.ascontiguousarray(query[b, sl]),
            "gamma": gamma, "beta": beta,
        })

    res = bass_utils.run_bass_kernel_spmd(
        nc, in_maps, core_ids=list(range(NCORES)), trace=TRACE
    )
    LAST_EXEC_NS = res.exec_time_ns

    out = np.empty((B, S, D), np.float32)
    for core in range(NCORES):
        b, half = core // 2, core % 2
        out[b, half * SH:(half + 1) * SH] = np.asarray(res.results[core]["out"])
    return out


# revision 17
# speedup vs baseline: 1.0859x; 1.0859x over previous
"""nn_MultiHeadAttention: fused MHA + residual + LayerNorm on 8 TRN2 NeuronCores.

Sharding: core = (batch b, query-half). Each core computes, for its batch:
  - Q projection for its 512 query rows, K/V projections for all 1024 keys
    (K/V work duplicated within a batch pair -> zero cross-core communication),
  - all 16 heads' attention for its query rows,
  - output projection + residual + LayerNorm for its rows.
Host concatenates the 8 [512, 1024] results into [4, 1024, 1024].

Schedule: the four V-projection passes run first (so PV can always drain the
exp stream), then K/Q for head pairs 0-3 (jg0).  Attention starts there:
scores are row-packed 2 heads across the PE's 64-row groups (concurrent),
exp runs on ACT, PV drains immediately.  The jg1 K/Q projections are split
into single-PSUM-tile sub-passes and interleaved between pairs 0-3 as PE
filler under the exp stream; pairs 4-7 follow with PE heaters to keep the
HAM clock gate warm.  O-projection + residual + LayerNorm close.

Queues: SP-HWDGE carries weights + output; ACT-HWDGE carries every
activation load, all issued before the first exp so the ACT engine is
exp-only afterwards; gpsimd carries only sum-broadcasts + resid/gamma/beta.
All matmuls f32r (1 cycle/row at N=512).
"""
import numpy as np

import concourse.bass as bass
import concourse.mybir as mybir
import concourse.tile as tile
from concourse import bacc, bass_utils

B, S, D, H, DK = 4, 1024, 1024, 16, 64
P = 128
SH = S // 2           # query rows per core
NC = D // P           # 8 chunks of 128 along any d-dimension
NP = H // 2           # 8 head pairs (one 128-dim chunk each)
NCORES = 8
EPS = 1e-6
f32 = mybir.dt.float32
f32r = mybir.dt.float32r

TRACE = False          # set by test.py to profile
LAST_EXEC_NS = None

_CACHE = {}


def _build():
    nc = bacc.Bacc("TRN2")
    xqT = nc.dram_tensor("xqT", [D, SH], f32, kind="ExternalInput")
    xkT = nc.dram_tensor("xkT", [D, S], f32, kind="ExternalInput")
    xvT = nc.dram_tensor("xvT", [D, S], f32, kind="ExternalInput")
    wq = nc.dram_tensor("wq", [D, D], f32, kind="ExternalInput")   # Wq.T  [in, out]
    wk = nc.dram_tensor("wk", [D, D], f32, kind="ExternalInput")
    wv = nc.dram_tensor("wv", [D, D], f32, kind="ExternalInput")
    wo = nc.dram_tensor("wo", [D, D], f32, kind="ExternalInput")   # Wo.T  [d, e]
    resid = nc.dram_tensor("resid", [SH, D], f32, kind="ExternalInput")
    gamma = nc.dram_tensor("gamma", [D], f32, kind="ExternalInput")
    beta = nc.dram_tensor("beta", [D], f32, kind="ExternalInput")
    out = nc.dram_tensor("out", [SH, D], f32, kind="ExternalOutput")

    with tile.TileContext(nc) as tc:
        with (
            tc.tile_pool(name="wpool", bufs=18) as wpool,
            tc.tile_pool(name="xp", bufs=13) as xp,
            tc.tile_pool(name="persist", bufs=1) as persist,
            tc.tile_pool(name="expp", bufs=6) as expp,
            tc.tile_pool(name="rp", bufs=2) as rp,
            tc.tile_pool(name="small", bufs=2) as small,
            tc.tile_pool(name="psum", bufs=3, space="PSUM") as psum,
            tc.tile_pool(name="pvps", bufs=2, space="PSUM") as pvps,
        ):
            # ---------------- persistent tiles ----------------
            kT = persist.tile([P, NC, S], f32r)       # [dim-in-pair, pair, sk]
            qT = persist.tile([P, NC, SH], f32r)      # [dim-in-pair, pair, sq]
            vt = persist.tile([P, NC, H, DK + 1], f32r)  # [sk-in-chunk, sk-chunk, (h, d|1)]
            xT = persist.tile([P, NC, SH], f32r)      # normalized attn out
            gb = persist.tile([P, 2, D], f32)         # gamma/beta broadcast
            eps_t = persist.tile([P, 1], f32)

            nc.vector.memset(eps_t, EPS)
            nc.vector.memset(vt[:, :, :, DK:DK + 1].bitcast(f32), 1.0)  # ones col

            def load_wh(w, i, ch, nm):
                """One [128, 512] weight col-half chunk (SP queue)."""
                wt = wpool.tile([P, SH], f32r, tag="w", name=f"{nm}{i}")
                nc.sync.dma_start(
                    wt, w[i * P:(i + 1) * P, ch * SH:(ch + 1) * SH].bitcast(f32r)
                )
                return wt

            def load_x(x, i, col0, nm):
                """One [128, 512] activation half-chunk (ACT queue)."""
                xc = xp.tile([P, SH], f32r, tag="x", name=nm)
                nc.scalar.dma_start(
                    xc, x[i * P:(i + 1) * P, col0:col0 + SH].bitcast(f32r)
                )
                return xc

            def big(nm="big"):
                return psum.tile([P, 2, SH], f32, tag="mm", name=nm)

            # ---------------- V projection (first: 4 passes) -------------
            # pass (scg, dh): vt[sk half scg, heads dh*8..dh*8+7]; xv half
            # chunks live across the two dh passes of their scg; wv halves
            # are re-loaded per pass (cheap on the weight queue).
            def v_pass(xv8, scg, dh):
                wvh = [load_wh(wv, i, dh, "wv") for i in range(NC)]
                ps_v = [big("psv") for _ in range(2)]
                for i in range(NC):
                    if dh == 0:
                        xv8.append(load_x(xvT, i, scg * SH, "xv"))
                    for sl in range(4):
                        nc.tensor.matmul(
                            ps_v[sl // 2][:, sl % 2, :],
                            xv8[i][:, sl * P:(sl + 1) * P], wvh[i],
                            start=(i == 0), stop=(i == NC - 1),
                        )
                for sl in range(4):
                    sc = scg * 4 + sl
                    nc.vector.tensor_copy(
                        vt[:, sc, dh * 8:(dh + 1) * 8, :DK],
                        ps_v[sl // 2][:, sl % 2, :].rearrange("p (h d) -> p h d", d=DK),
                    )

            for scg in range(2):
                xv8 = []
                v_pass(xv8, scg, 0)
                v_pass(xv8, scg, 1)

            # ---------------- K / Q projection passes ----------------
            # kq_pass computes out-cols of the projection for head pairs
            # js (2 per PSUM tile) from source column half `half`.
            def kq_pass(wget, xget, js, dst, dsl):
                ps2 = [big("pskq") for _ in range(len(js) // 2)]
                for i in range(NC):
                    xc = xget(i)
                    for jj, j in enumerate(js):
                        nc.tensor.matmul(
                            ps2[jj // 2][:, jj % 2, :],
                            wget(i)[:, (j % 4) * P:(j % 4 + 1) * P], xc,
                            start=(i == 0), stop=(i == NC - 1),
                        )
                for jj, j in enumerate(js):
                    nc.vector.tensor_copy(dst[:, j, dsl], ps2[jj // 2][:, jj % 2, :])

            def wcache(w, jg, nm):
                tiles = {}

                def get(i):
                    if i not in tiles:
                        tiles[i] = load_wh(w, i, jg, nm)
                    return tiles[i]
                return get

            def xcache(x, half):
                tiles = {}

                def get(i):
                    if i not in tiles:
                        tiles[i] = load_x(x, i, half * SH, "xkq")
                    return tiles[i]
                return get

            # jg0: head pairs 0-3, both sk halves, then qT pairs 0-3
            wk0 = wcache(wk, 0, "wk")
            kq_pass(wk0, xcache(xkT, 0), (0, 1, 2, 3), kT, slice(0, SH))
            kq_pass(wk0, xcache(xkT, 1), (0, 1, 2, 3), kT, slice(SH, S))
            wq0 = wcache(wq, 0, "wq")
            kq_pass(wq0, xcache(xqT, 0), (0, 1, 2, 3), qT, slice(None))

            # ---------------- attention ----------------
            pv_state = {}

            def sc_exp_cp(p, cp):
                """Row-packed scores for heads (2p, 2p+1), sk chunks
                (2cp, 2cp+1), then exp. Returns (etA, etB)."""
                ps2 = [big("scps") for _ in range(2)]
                for k in range(2):
                    c = 2 * cp + k
                    for a in range(2):
                        nc.tensor.matmul(
                            ps2[a][:, k, :],
                            kT[a * DK:(a + 1) * DK, p, c * P:(c + 1) * P],
                            qT[a * DK:(a + 1) * DK, p, :],
                            start=True, stop=True,
                        )
                ets = []
                for a in range(2):
                    et = expp.tile([P, 2, SH], f32r, tag="e", name="et")
                    nc.scalar.activation(
                        out=et, in_=ps2[a],
                        func=mybir.ActivationFunctionType.Exp,
                        scale=1.0 / np.sqrt(np.float32(DK)),
                    )
                    ets.append(et)
                return ets

            def pv_cp(h, cp, et):
                """Accumulate PV for head h over sk chunks (2cp, 2cp+1);
                normalize after the last chunk."""
                if h not in pv_state:
                    pv_state[h] = pvps.tile([P, SH], f32, tag="pv", name="pvt")
                pvt = pv_state[h]
                for k in range(2):
                    c = 2 * cp + k
                    nc.tensor.matmul(
                        pvt[:DK + 1, :], vt[:, c, h, :], et[:, k, :],
                        start=(c == 0), stop=(c == NC - 1),
                    )
                if cp != 3:
                    return
                del pv_state[h]
                sums_raw = small.tile([1, SH], f32, tag="sums_raw", name="sums_raw",
                                      bufs=1)
                nc.vector.tensor_copy(sums_raw, pvt[DK:DK + 1, :])
                sums = small.tile([1, SH], f32, tag="sums", name="sums")
                # approx reciprocal needs an SBUF input (bit-trick path)
                nc.vector.reciprocal_approx_fast(sums, sums_raw)
                rbc = small.tile([DK, SH], f32, tag="rbc", name="rbc", bufs=1)
                nc.gpsimd.partition_broadcast(rbc, sums)
                a, p = h % 2, h // 2
                nc.vector.tensor_mul(
                    out=xT[a * DK:(a + 1) * DK, p, :], in0=pvt[:DK, :], in1=rbc
                )

            def pair(p):
                for cp in range(4):
                    etA, etB = sc_exp_cp(p, cp)
                    pv_cp(2 * p, cp, etA)
                    pv_cp(2 * p + 1, cp, etB)

            def heater():
                hp = big("heat")
                nc.tensor.matmul(
                    hp[:1, 0, :], qT[0:1, 0, 0:1], qT[0:1, 0, :],
                    start=True, stop=True,
                )

            # pairs 0-3 with the jg1 K/Q sub-passes as PE filler under exp;
            # pairs 4-7 with heaters to keep HAM warm.
            wk1 = wcache(wk, 1, "wk")
            wq1 = wcache(wq, 1, "wq")
            pair(0)
            xk1h0 = xcache(xkT, 0)
            kq_pass(wk1, xk1h0, (4, 5), kT, slice(0, SH))
            kq_pass(wk1, xk1h0, (6, 7), kT, slice(0, SH))
            pair(1)
            xk1h1 = xcache(xkT, 1)
            kq_pass(wk1, xk1h1, (4, 5), kT, slice(SH, S))
            kq_pass(wk1, xk1h1, (6, 7), kT, slice(SH, S))
            pair(2)
            xq1 = xcache(xqT, 0)
            kq_pass(wq1, xq1, (4, 5), qT, slice(None))
            kq_pass(wq1, xq1, (6, 7), qT, slice(None))
            pair(3)
            for p in range(4, NP):
                pair(p)
                if p < NP - 1:
                    heater()

            # ---------------- output projection + residual + LN ----------
            wo16 = [[load_wh(wo, i, eh, "wo") for eh in range(2)] for i in range(NC)]
            for i, t in enumerate((gamma, beta)):
                nc.gpsimd.dma_start(
                    gb[:, i, :], bass.AP(tensor=t, offset=0, ap=[[0, P], [1, D]])
                )
            for scc in range(4):
                rc = rp.tile([P, D], f32, tag="r", name="rc")
                nc.gpsimd.dma_start(rc, resid[scc * P:(scc + 1) * P, :])
                ps_o = big("pso")
                for dc in range(NC):
                    for eh in range(2):
                        nc.tensor.matmul(
                            ps_o[:, eh, :],
                            xT[:, dc, scc * P:(scc + 1) * P],
                            wo16[dc][eh],
                            start=(dc == 0), stop=(dc == NC - 1),
                        )
                xl = rc  # LN runs in-place on the residual tile
                nc.vector.tensor_add(
                    out=xl, in0=ps_o.rearrange("p a b -> p (a b)"), in1=rc
                )
                stats = small.tile([P, 2, nc.vector.BN_STATS_DIM], f32, tag="stats",
                                   name="stats")
                for i in range(2):
                    nc.vector.bn_stats(stats[:, i, :], xl[:, i * SH:(i + 1) * SH])
                mv = small.tile([P, nc.vector.BN_AGGR_DIM], f32, tag="mv", name="mv")
                nc.vector.bn_aggr(mv, stats)
                std = small.tile([P, 1], f32, tag="std", name="std")
                nc.scalar.activation(
                    out=std, in_=mv[:, 1:2],
                    func=mybir.ActivationFunctionType.Sqrt,
                    bias=eps_t, scale=1.0,
                )
                rstd = small.tile([P, 1], f32, tag="rstd", name="rstd")
                nc.vector.reciprocal_approx_fast(rstd, std)
                nc.vector.tensor_scalar(
                    out=xl, in0=xl, scalar1=mv[:, 0:1], scalar2=rstd,
                    op0=mybir.AluOpType.subtract, op1=mybir.AluOpType.mult,
                )
                nc.vector.tensor_mul(out=xl, in0=xl, in1=gb[:, 0, :])
                nc.vector.tensor_add(out=xl, in0=xl, in1=gb[:, 1, :])
                nc.sync.dma_start(out[scc * P:(scc + 1) * P, :], xl)

    nc.compile()
    return nc


def kernel(query, key, value, Wq, Wk, Wv, Wo, ln_gamma, ln_beta):
    global LAST_EXEC_NS
    if "nc" not in _CACHE:
        _CACHE["nc"] = _build()
    nc = _CACHE["nc"]

    query = np.asarray(query, np.float32)
    key = np.asarray(key, np.float32)
    value = np.asarray(value, np.float32)
    wqT = np.ascontiguousarray(np.asarray(Wq, np.float32).T)
    wkT = np.ascontiguousarray(np.asarray(Wk, np.float32).T)
    wvT = np.ascontiguousarray(np.asarray(Wv, np.float32).T)
    woT = np.ascontiguousarray(np.asarray(Wo, np.float32).T)
    gamma = np.ascontiguousarray(np.asarray(ln_gamma, np.float32))
    beta = np.ascontiguousarray(np.asarray(ln_beta, np.float32))

    in_maps = []
    for core in range(NCORES):
        b, half = core // 2, core % 2
        sl = slice(half * SH, (half + 1) * SH)
        in_maps.append({
            "xqT": np.ascontiguousarray(query[b].T[:, sl]),
            "xkT": np.ascontiguousarray(key[b].T),
            "xvT": np.ascontiguousarray(value[b].T),
            "wq": wqT, "wk": wkT, "wv": wvT, "wo": woT,
            "resid": np.ascontiguousarray(query[b, sl]),
            "gamma": gamma, "beta": beta,
        })

    res = bass_utils.run_bass_kernel_spmd(
        nc, in_maps, core_ids=list(range(NCORES)), trace=TRACE
    )
    LAST_EXEC_NS = res.exec_time_ns

    out = np.empty((B, S, D), np.float32)
    for core in range(NCORES):
        b, half = core // 2, core % 2
        out[b, half * SH:(half + 1) * SH] = np.asarray(res.results[core]["out"])
    return out
